# revision 21
# baseline (speedup 1.0000x reference)
"""Slot-attention module Bass/Tile kernel (nn_AttentionModule_39084202394083).

kernel(**inputs) takes FULL unsharded inputs, shards batch B=64 across 8
NeuronCores (8 batch elements per core), runs a Bass/Tile kernel per core,
and gathers the FULL output [S=8, B=64, D=256] fp32.

Device algorithm:
- All matmuls bf16 (fp32 PSUM accumulation); correctness gate is 2e-2.
- Input LN is folded into the attention algebra: per-token stats come from
  bn_stats on natural-layout x; the (x-m)*r normalize is never
  materialized. The mean enters dots via an augmented contraction row
  (mT row x -colsum(M_k)) and enters updates via an extra value column;
  the 1/std scale r enters dots pre-exp and updates via the softmax
  weights. Host passes a pre-transposed bf16 x (stationary operand).
- wq/wk are pre-multiplied on host (W_qk = wk @ wq.T * D^-0.5) so q/k are
  never materialized; logits for 128-token chunks are x_chunk.T @ (W_qk@snT).
- Softmax over the slot axis runs in token-major layout [128, 32*8].
- LN rsqrt is computed on VectorE (bit-trick seed + 2 Newton steps) so
  ScalarE only ever uses the exp/tanh table set (no table thrash).
- GRU + MLP run in transposed layout [D, rows] with weights stationary;
  sigmoid is computed as 0.5 + 0.5*tanh(x/2).
"""

import numpy as np
import ml_dtypes

BF16 = ml_dtypes.bfloat16


def _install_ntff_hook_shim():
    """Provide antenv.axon_hooks in containers whose antenv stub lacks it,
    backed by direct ctypes calls into libaxon_pjrt.so (same recipe as
    trn_agent_boot). Enables trace=True HW timing via run_bass_kernel_spmd."""
    import sys, types, ctypes, contextlib, os
    try:
        from antenv.axon_hooks import get_axon_ntff_profile_hook  # noqa
        return
    except ImportError:
        pass
    so_path = "/opt/axon/libaxon_pjrt.so"
    if not os.path.exists(so_path):
        return
    try:
        lib = ctypes.CDLL(so_path)
        if not hasattr(lib, "axon_start_nrt_profile"):
            return
        lib.axon_start_nrt_profile.argtypes = [
            ctypes.POINTER(ctypes.c_int64), ctypes.c_size_t]
        lib.axon_start_nrt_profile.restype = ctypes.c_int64
        lib.axon_stop_nrt_profile.argtypes = [ctypes.c_char_p]
        lib.axon_stop_nrt_profile.restype = ctypes.c_int64
    except OSError:
        return

    @contextlib.contextmanager
    def _hook(output_dir, device_ids):
        import jax
        jax.devices()
        if device_ids:
            ids = (ctypes.c_int64 * len(device_ids))(*device_ids)
            rc = lib.axon_start_nrt_profile(ids, len(device_ids))
        else:
            rc = lib.axon_start_nrt_profile(None, 0)
        if rc != 0:
            raise RuntimeError(f"axon_start_nrt_profile rc={rc}")
        try:
            yield
        finally:
            n = lib.axon_stop_nrt_profile(str(output_dir).encode())
            if n < 0:
                raise RuntimeError(f"axon_stop_nrt_profile rc={n}")

    mod = types.ModuleType("antenv.axon_hooks")
    mod.get_axon_ntff_profile_hook = lambda: _hook
    mod.set_axon_ntff_profile_hook = lambda h: None
    import antenv
    antenv.axon_hooks = mod
    sys.modules["antenv.axon_hooks"] = mod


_install_ntff_hook_shim()

NUM_SLOTS = 8
DIM = 256
HID = 1024
ITERS = 3
LN_EPS = 1e-05

N_CORES = 8
BL = 8          # batch elements per core
GB = 2          # batch elements per phase-group
NGROUPS = BL // GB
N = 4096        # tokens
NCH = N // 128  # token chunks of 128
ROWS = GB * NUM_SLOTS  # slot-rows per group (16)

_RSQRT_MAGIC = 0x5F3759DF


def _build_nc():
    import concourse.bass as bass
    import concourse.bacc as bacc
    import concourse.tile as tile
    from concourse import mybir

    f32 = mybir.dt.float32
    bf16 = mybir.dt.bfloat16
    i32 = mybir.dt.int32
    AX = mybir.AxisListType
    OP = mybir.AluOpType
    ACT = mybir.ActivationFunctionType

    nc = bacc.Bacc("TRN2", target_bir_lowering=False)

    xnat = nc.dram_tensor("xnat", [BL, 128, NCH, DIM], bf16, kind="ExternalInput")
    xt = nc.dram_tensor("xt", [BL, 2, 128, N], bf16, kind="ExternalInput")
    slots_in = nc.dram_tensor("slots_in", [BL * NUM_SLOTS, DIM], f32,
                              kind="ExternalInput")
    wqkT_d = nc.dram_tensor("wqkT", [2, 128, DIM], bf16, kind="ExternalInput")
    w1rT_d = nc.dram_tensor("w1rT", [2, 128, 1], bf16, kind="ExternalInput")
    wv_d = nc.dram_tensor("wv", [2, 128, DIM], bf16, kind="ExternalInput")
    wv1_d = nc.dram_tensor("wv1", [1, DIM], f32, kind="ExternalInput")
    wih_d = nc.dram_tensor("wih", [2, 128, 3 * DIM], bf16, kind="ExternalInput")
    whh_d = nc.dram_tensor("whh", [2, 128, 3 * DIM], bf16, kind="ExternalInput")
    mlp1_d = nc.dram_tensor("mlp1", [2, 128, HID], bf16, kind="ExternalInput")
    mlp2_d = nc.dram_tensor("mlp2", [8, 128, DIM], bf16, kind="ExternalInput")
    bsum_d = nc.dram_tensor("bsum", [128, 4], f32, kind="ExternalInput")
    bihn_d = nc.dram_tensor("bihn", [128, 2], f32, kind="ExternalInput")
    bhhn_d = nc.dram_tensor("bhhn", [128, 2], f32, kind="ExternalInput")
    out_d = nc.dram_tensor("out", [BL * NUM_SLOTS, DIM], f32,
                           kind="ExternalOutput")

    with tile.TileContext(nc) as tc:
        with (
            tc.tile_pool(name="consts", bufs=1) as consts,
            tc.tile_pool(name="xtp", bufs=1) as xtp,
            tc.tile_pool(name="vpool", bufs=1) as vpool,
            tc.tile_pool(name="perb", bufs=1) as perb,
            tc.tile_pool(name="xch", bufs=3) as xch_pool,
            tc.tile_pool(name="stats", bufs=2) as stats,
            tc.tile_pool(name="sm", bufs=2) as sm,
            tc.tile_pool(name="ep", bufs=2) as ep,
            tc.tile_pool(name="slp", bufs=1) as slp,
            tc.tile_pool(name="ps", bufs=1, space="PSUM") as ps,
            tc.tile_pool(name="ps2", bufs=2, space="PSUM") as ps2,
        ):
            from concourse.masks import make_identity
            ident = consts.tile([128, 128], bf16, tag="ident")
            make_identity(nc, ident[:])
            identf = consts.tile([128, 128], f32, tag="identf")
            make_identity(nc, identf[:])

            wqk_sb = consts.tile([128, 2, DIM], bf16, tag="wqk")
            w1r_sb = consts.tile([128, 2, 1], bf16, tag="w1r")
            wv_sb = consts.tile([128, 2, DIM], bf16, tag="wv")
            wih_sb = consts.tile([128, 2, 3 * DIM], bf16, tag="wih")
            whh_sb = consts.tile([128, 2, 3 * DIM], bf16, tag="whh")
            mlp1_sb = consts.tile([128, 2, HID], bf16, tag="mlp1")
            mlp2_sb = consts.tile([128, 8, DIM], bf16, tag="mlp2")
            for _kt in range(2):
                nc.sync.dma_start(wqk_sb[:, _kt, :], wqkT_d[_kt])
                nc.sync.dma_start(w1r_sb[:, _kt, :], w1rT_d[_kt])
                nc.sync.dma_start(wv_sb[:, _kt, :], wv_d[_kt])
                nc.sync.dma_start(wih_sb[:, _kt, :], wih_d[_kt])
                nc.sync.dma_start(whh_sb[:, _kt, :], whh_d[_kt])
                nc.sync.dma_start(mlp1_sb[:, _kt, :], mlp1_d[_kt])
            for _kt in range(8):
                nc.sync.dma_start(mlp2_sb[:, _kt, :], mlp2_d[_kt])
            bsum_sb = consts.tile([128, 4], f32, tag="bsum")
            nc.sync.dma_start(bsum_sb[:], bsum_d[:, :])
            bihn_sb = consts.tile([128, 2], f32, tag="bihn")
            nc.sync.dma_start(bihn_sb[:], bihn_d[:, :])
            bhhn_sb = consts.tile([128, 2], f32, tag="bhhn")
            nc.sync.dma_start(bhhn_sb[:], bhhn_d[:, :])
            ones_row = consts.tile([1, 128], bf16, tag="ones_row")
            nc.gpsimd.memset(ones_row[:], 1.0)
            wv1_sb = consts.tile([128, DIM], f32, tag="wv1")
            _w = wv1_d[:, :]
            wv1_bc = bass.AP(tensor=_w.tensor, offset=_w.offset,
                             ap=[[0, 128]] + list(_w.ap)[1:])
            nc.sync.dma_start(wv1_sb[:], wv1_bc)

            def bcast8(ap2):
                """[128, NCH] AP -> [128, NCH, 8] stride-0 broadcast."""
                return bass.AP(tensor=ap2.tensor, offset=ap2.offset,
                               ap=list(ap2.ap) + [[0, 8]])

            def newton_rsqrt(eng, ve_ap, y_ap, u_ap):
                """y = 1/sqrt(ve), bit-trick seed + 2 Newton steps (f32)."""
                vi = ve_ap.bitcast(i32)
                yi = y_ap.bitcast(i32)
                eng.tensor_scalar(yi, vi, 1, None, OP.arith_shift_right)
                eng.tensor_scalar(yi, yi, -1, _RSQRT_MAGIC, OP.mult, OP.add)
                for _ in range(2):
                    eng.tensor_tensor(u_ap, y_ap, y_ap, OP.mult)
                    eng.tensor_tensor(u_ap, u_ap, ve_ap, OP.mult)
                    eng.tensor_scalar(u_ap, u_ap, -0.5, 1.5, OP.mult, OP.add)
                    eng.tensor_tensor(y_ap, y_ap, u_ap, OP.mult)

            def transpose_in(dst, src, rows):
                """dst [128, 2, rows] bf16 <- src [rows, 256] bf16."""
                for kt in range(2):
                    tp = ps2.tile([128, 128], bf16, tag="tp")
                    nc.tensor.transpose(tp[:128, :rows],
                                        src[:, kt * 128:(kt + 1) * 128],
                                        ident[0:rows, 0:rows])
                    nc.vector.tensor_copy(dst[:, kt, :], tp[:128, :rows])

            def transpose_out(dst, srcT, rows):
                """dst [rows, 256] f32 <- srcT [128, 2, rows] f32."""
                for kt in range(2):
                    tp = ps2.tile([128, 128], f32, tag="tp")
                    nc.tensor.transpose(tp[:rows, :128], srcT[:, kt, :],
                                        identf[:, :])
                    nc.vector.tensor_copy(dst[:, kt * 128:(kt + 1) * 128],
                                          tp[:rows, :128])

            def phase_sv(g, gi):
                """Load xt, compute stats + v for group g (slot gi)."""
                st = {"xt": [], "y": [], "mr2": [], "v": []}
                for b in range(GB):
                    gb = g * GB + b
                    tiles = []
                    for kt in range(2):
                        t = xtp.tile([128, N], bf16, tag=f"xt{gi}_{b}_{kt}")
                        nc.sync.dma_start(t[:], xt[gb, kt])
                        tiles.append(t)
                    st["xt"].append(tiles)

                    mv = stats.tile([128, NCH, 2], f32, tag="mv")
                    for j4 in range(4):
                        xb = xch_pool.tile([128, 8, DIM], bf16, tag="xc")
                        nc.sync.dma_start(xb[:],
                                          xnat[gb, :, j4 * 8:(j4 + 1) * 8, :])
                        for jj in range(8):
                            c = j4 * 8 + jj
                            st6 = sm.tile([128, 6], f32, tag="st6")
                            nc.vector.bn_stats(st6[:], xb[:, jj, :])
                            nc.vector.bn_aggr(mv[:, c, :], st6[:])
                    ve = stats.tile([128, NCH], f32, tag="ve")
                    nc.vector.tensor_scalar(ve[:], mv[:, :, 1], LN_EPS, None,
                                            OP.add)
                    y = perb.tile([128, NCH], f32, tag=f"y{gi}{b}")
                    u = sm.tile([128, NCH], f32, tag="u")
                    newton_rsqrt(nc.vector, ve[:], y[:], u[:])
                    st["y"].append(y)
                    srec = sm.tile([128, NCH], f32, tag="srec")
                    nc.vector.tensor_tensor(srec[:], ve[:], y[:], OP.mult)
                    m_bf = stats.tile([128, NCH], bf16, tag="mbf")
                    nc.gpsimd.tensor_copy(m_bf[:], mv[:, :, 0])
                    mr2 = perb.tile([128, NCH], f32, tag=f"mr2{gi}{b}")
                    nc.vector.tensor_tensor(mr2[:], mv[:, :, 0], y[:],
                                            OP.mult)
                    st["mr2"].append(mr2)

                    vt = vpool.tile([128, NCH, DIM + 2], bf16, tag=f"v{gi}{b}")
                    nc.gpsimd.tensor_copy(vt[:, :, DIM], srec[:])
                    nc.gpsimd.tensor_copy(vt[:, :, DIM + 1], m_bf[:])
                    for cc in range(0, NCH, 2):
                        vps = ps2.tile([128, 2, DIM], f32, tag="vp")
                        for j in range(2):
                            c = cc + j
                            for kt in range(2):
                                nc.tensor.matmul(
                                    vps[:, j, :],
                                    tiles[kt][:, c * 128:(c + 1) * 128],
                                    wv_sb[:, kt, :],
                                    start=(kt == 0), stop=(kt == 1))
                        nc.scalar.activation(vt[:, cc:cc + 2, 0:DIM], vps[:],
                                             ACT.Copy)
                    st["v"].append(vt)

                sl_nat = slp.tile([ROWS, DIM], f32, tag=f"slnat{gi}")
                nc.sync.dma_start(sl_nat[:],
                                  slots_in[g * ROWS:(g + 1) * ROWS, :])
                sl_bf0 = slp.tile([ROWS, DIM], bf16, tag=f"slbf0{gi}")
                nc.vector.tensor_copy(sl_bf0[:], sl_nat[:])
                slT_bf = slp.tile([128, 2, ROWS], bf16, tag=f"slTbf{gi}")
                transpose_in(slT_bf, sl_bf0, ROWS)
                st["sl_nat"] = sl_nat
                st["slT_bf"] = slT_bf
                return st

            def ln0(src, rows, out_bf, eng=None):
                """out_bf [rows, 256] bf16 = LN0(src [rows, 256])."""
                eng = nc.vector
                st6 = sm.tile([rows, 6], f32, tag="st6b")
                nc.vector.bn_stats(st6[:], src[:])
                mv2 = sm.tile([rows, 2], f32, tag="mv2")
                nc.vector.bn_aggr(mv2[:], st6[:])
                ve2 = sm.tile([rows, 1], f32, tag="ve2")
                eng.tensor_scalar(ve2[:], mv2[:, 1:2], LN_EPS, None, OP.add)
                y2 = sm.tile([rows, 1], f32, tag="y2")
                u2 = sm.tile([rows, 1], f32, tag="u2")
                newton_rsqrt(eng, ve2[:], y2[:], u2[:])
                nc.vector.tensor_scalar(out_bf[:], src[:], mv2[:, 0:1],
                                        y2[:], OP.subtract, OP.mult)

            def iteration(g, gi, it, st):
                # ---- P1: sn = LN0(slots), snT ----
                sn_bf = sm.tile([ROWS, DIM], bf16, tag="snbf")
                ln0(st["sl_nat"], ROWS, sn_bf)
                snT = sm.tile([128, 2, ROWS], bf16, tag="snT")
                transpose_in(snT, sn_bf, ROWS)
                # ---- P3: M_k, Mksum broadcast ----
                mk_ps = ps2.tile([128, 2, ROWS], f32, tag="tp")
                for mt in range(2):
                    for kt in range(2):
                        nc.tensor.matmul(
                            mk_ps[:, mt, :],
                            wqk_sb[:, kt, mt * 128:(mt + 1) * 128],
                            snT[:, kt, :], start=(kt == 0), stop=(kt == 1))
                mk_sb = sm.tile([128, 2, ROWS], bf16, tag="mksb")
                nc.vector.tensor_copy(mk_sb[:], mk_ps[:])
                mn_ps = ps2.tile([1, ROWS], f32, tag="tp")
                for kt in range(2):
                    nc.tensor.matmul(mn_ps[:], w1r_sb[:, kt, :],
                                     snT[:, kt, :],
                                     start=(kt == 0), stop=(kt == 1))
                mks_bf = sm.tile([1, ROWS], bf16, tag="mksbf")
                nc.vector.tensor_copy(mks_bf[:], mn_ps[:])
                bc_ps = ps2.tile([128, ROWS], f32, tag="tp")
                nc.tensor.matmul(bc_ps[:], ones_row[:], mks_bf[:],
                                 start=True, stop=True)
                mkbc = sm.tile([128, ROWS], f32, tag="mkbc")
                nc.vector.tensor_copy(mkbc[:], bc_ps[:])

                # ---- P4: attention per batch ----
                upd_ps = ps2.tile([128, DIM + 2], f32, tag="vp")
                for b in range(GB):
                    brow = b * NUM_SLOTS
                    bsl = slice(brow, brow + NUM_SLOTS)
                    dt_ps = ps2.tile([128, NCH, NUM_SLOTS], f32, tag="dt")
                    for c in range(NCH):
                        cs = slice(c * 128, (c + 1) * 128)
                        nc.tensor.matmul(dt_ps[:, c, :],
                                         st["xt"][b][0][:, cs],
                                         mk_sb[:, 0, bsl],
                                         start=True, stop=False)
                        nc.tensor.matmul(dt_ps[:, c, :],
                                         st["xt"][b][1][:, cs],
                                         mk_sb[:, 1, bsl],
                                         start=False, stop=True)
                    _mk = mkbc[:, bsl]
                    mk_bc = bass.AP(tensor=_mk.tensor, offset=_mk.offset,
                                    ap=[_mk.ap[0], [0, NCH], _mk.ap[1]])
                    t2r = ep.tile([128, NCH, 8], f32, tag="t2r")
                    nc.vector.tensor_tensor(t2r[:], bcast8(st["mr2"][b][:]),
                                            mk_bc, OP.mult)
                    tmp_e = ep.tile([128, NCH, 8], f32, tag="tmpe")
                    nc.vector.tensor_tensor(tmp_e[:], dt_ps[:],
                                            bcast8(st["y"][b][:]), OP.mult)
                    et_f = ep.tile([128, NCH, 8], f32, tag="etf")
                    nc.vector.tensor_tensor(et_f[:], tmp_e[:], t2r[:],
                                            OP.subtract)
                    eT = ep.tile([128, NCH, 8], bf16, tag="eT")
                    nc.scalar.activation(
                        eT[:].rearrange("p a b -> p (a b)"),
                        et_f[:].rearrange("p a b -> p (a b)"), ACT.Exp)
                    cs_f = sm.tile([128, NCH], f32, tag="csf")
                    nc.vector.tensor_reduce(cs_f[:], eT[:], axis=AX.X,
                                            op=OP.add)
                    rc = sm.tile([128, NCH], f32, tag="rc")
                    nc.vector.reciprocal(rc[:], cs_f[:])
                    rcr = sm.tile([128, NCH], f32, tag="rcr")
                    nc.vector.tensor_tensor(rcr[:], rc[:], st["y"][b][:],
                                            OP.mult)
                    wpT = ep.tile([128, NCH, 8], bf16, tag="wpT")
                    nc.vector.tensor_tensor(wpT[:], eT[:], bcast8(rcr[:]),
                                            OP.mult)
                    for c in range(NCH):
                        nc.tensor.matmul(upd_ps[32 * b:32 * b + NUM_SLOTS, :],
                                         wpT[:, c, :], st["v"][b][:, c, :],
                                         start=(c == 0), stop=(c == NCH - 1),
                                         tile_position=(0, 32 * b))

                # ---- P5: updates fixup + GRU ----
                upd_bf = sm.tile([128, DIM], bf16, tag="updbf")
                rn_all = sm.tile([128, 1], f32, tag="rn")
                murn_all = sm.tile([128, 1], f32, tag="murn")
                tfix_all = sm.tile([128, DIM], f32, tag="tfix")
                nc.vector.memset(upd_bf[:], 0.0)
                for b in range(GB):
                    psl = slice(32 * b, 32 * b + NUM_SLOTS)
                    nc.vector.reciprocal(rn_all[psl],
                                         upd_ps[psl, DIM:DIM + 1])
                    nc.vector.tensor_tensor(
                        murn_all[psl], upd_ps[psl, DIM + 1:DIM + 2],
                        rn_all[psl], OP.mult)
                    nc.vector.tensor_scalar(tfix_all[psl], wv1_sb[psl],
                                            murn_all[psl], None, OP.mult)
                    nc.vector.scalar_tensor_tensor(
                        upd_bf[psl], upd_ps[psl, 0:DIM], rn_all[psl],
                        tfix_all[psl], OP.mult, OP.subtract)
                updT = sm.tile([128, 2, 128], bf16, tag="updT")
                for kt in range(2):
                    tpu = ps2.tile([128, 128], bf16, tag="tp")
                    nc.tensor.transpose(tpu[:],
                                        upd_bf[:, kt * 128:(kt + 1) * 128],
                                        ident[:, :])
                    nc.vector.tensor_copy(updT[:, kt, :], tpu[:])

                gx_ps = ps.tile([128, 6, ROWS], f32, tag="gx")
                gh_ps = ps.tile([128, 6, ROWS], f32, tag="gh")
                for t in range(6):
                    tsl = slice(t * 128, (t + 1) * 128)
                    for kt in range(2):
                        nc.tensor.matmul(
                            gx_ps[:, t, :], wih_sb[:, kt, tsl],
                            updT[:, kt, :]
                            .rearrange("p (b s) -> p b s", s=32)
                            [:, 0:GB, 0:NUM_SLOTS],
                            start=(kt == 0), stop=(kt == 1))
                        nc.tensor.matmul(
                            gh_ps[:, t, :], whh_sb[:, kt, tsl],
                            st["slT_bf"][:, kt, :],
                            start=(kt == 0), stop=(kt == 1))
                gh_sb = sm.tile([128, 6, ROWS], f32, tag="ghsb")
                nc.scalar.activation(
                    gh_sb[:].rearrange("p a b -> p (a b)"),
                    gh_ps[:].rearrange("p a b -> p (a b)"),
                    ACT.Copy)
                rzp = sm.tile([128, 4, ROWS], f32, tag="rzp")
                for t in range(4):
                    nc.vector.scalar_tensor_tensor(
                        rzp[:, t, :], gx_ps[:, t, :], bsum_sb[:, t:t + 1],
                        gh_sb[:, t, :], OP.add, OP.add)
                rz_t = sm.tile([128, 4, ROWS], f32, tag="rzt")
                nc.scalar.activation(
                    rz_t[:].rearrange("p a b -> p (a b)"),
                    rzp[:].rearrange("p a b -> p (a b)"), ACT.Tanh, scale=0.5)
                rz = sm.tile([128, 4, ROWS], bf16, tag="rz")
                nc.vector.tensor_scalar(rz[:], rz_t[:], 0.5, 0.5, OP.mult,
                                        OP.add)
                hnp = sm.tile([128, 2, ROWS], f32, tag="hnp")
                for t in range(2):
                    nc.vector.tensor_scalar(hnp[:, t, :], gh_sb[:, 4 + t, :],
                                            bhhn_sb[:, t:t + 1], None, OP.add)
                t2t = sm.tile([128, 2, ROWS], f32, tag="t2t")
                nc.vector.tensor_tensor(t2t[:], rz[:, 0:2, :], hnp[:],
                                        OP.mult)
                npre = sm.tile([128, 2, ROWS], f32, tag="npre")
                for t in range(2):
                    nc.vector.scalar_tensor_tensor(
                        npre[:, t, :], gx_ps[:, 4 + t, :],
                        bihn_sb[:, t:t + 1], t2t[:, t, :], OP.add, OP.add)
                nT = sm.tile([128, 2, ROWS], bf16, tag="nT")
                nc.scalar.activation(
                    nT[:].rearrange("p a b -> p (a b)"),
                    npre[:].rearrange("p a b -> p (a b)"), ACT.Tanh)
                dd = sm.tile([128, 2, ROWS], f32, tag="dd")
                nc.vector.tensor_tensor(dd[:], st["slT_bf"][:], nT[:],
                                        OP.subtract)
                zd = sm.tile([128, 2, ROWS], f32, tag="zd")
                nc.vector.tensor_tensor(zd[:], rz[:, 2:4, :], dd[:], OP.mult)
                smidT_f = slp.tile([128, 2, ROWS], f32, tag=f"smidTf{gi}")
                nc.vector.tensor_tensor(smidT_f[:], nT[:], zd[:], OP.add)

                # ---- P6: LN-ff + MLP + residual ----
                smid_nat = sm.tile([ROWS, DIM], f32, tag="smidnat")
                transpose_out(smid_nat, smidT_f, ROWS)
                ffn_bf = sm.tile([ROWS, DIM], bf16, tag="ffnbf")
                ln0(smid_nat, ROWS, ffn_bf, eng=nc.gpsimd)
                ffnT = sm.tile([128, 2, ROWS], bf16, tag="ffnT")
                transpose_in(ffnT, ffn_bf, ROWS)
                h1_ps = ps2.tile([128, 8, ROWS], f32, tag="vp")
                for t in range(8):
                    tsl = slice(t * 128, (t + 1) * 128)
                    for kt in range(2):
                        nc.tensor.matmul(h1_ps[:, t, :], mlp1_sb[:, kt, tsl],
                                         ffnT[:, kt, :],
                                         start=(kt == 0), stop=(kt == 1))
                h1_bf = sm.tile([128, 8, ROWS], bf16, tag="h1bf")
                nc.vector.tensor_scalar(
                    h1_bf[:].rearrange("p a b -> p (a b)"),
                    h1_ps[:].rearrange("p a b -> p (a b)"), 0.0, None, OP.max)
                o2_ps = ps.tile([128, 2, ROWS], f32, tag="gx")
                for mt in range(2):
                    msl = slice(mt * 128, (mt + 1) * 128)
                    for kt in range(8):
                        nc.tensor.matmul(o2_ps[:, mt, :], mlp2_sb[:, kt, msl],
                                         h1_bf[:, kt, :],
                                         start=(kt == 0), stop=(kt == 7))
                slT_new_f = slp.tile([128, 2, ROWS], f32, tag=f"slTnf{gi}")
                nc.vector.tensor_tensor(slT_new_f[:], smidT_f[:], o2_ps[:],
                                        OP.add)
                new_nat = slp.tile([ROWS, DIM], f32, tag=f"slnat2{gi}")
                transpose_out(new_nat, slT_new_f, ROWS)
                st["sl_nat"] = new_nat
                if it < ITERS - 1:
                    slT_bf = slp.tile([128, 2, ROWS], bf16, tag=f"slTbf{gi}")
                    nc.vector.tensor_copy(slT_bf[:], slT_new_f[:])
                    st["slT_bf"] = slT_bf
                else:
                    nc.sync.dma_start(out_d[g * ROWS:(g + 1) * ROWS, :],
                                      new_nat[:])

            for g in range(NGROUPS):
                st = phase_sv(g, g % 2)
                for it in range(ITERS):
                    iteration(g, g % 2, it, st)

    nc.compile()
    return nc


def _host_prep(inputs, slots, wq, wk, wv, w_ih, b_ih, w_hh, b_hh,
               mlp_w1, mlp_w2, g_in, g_sl, g_ff):
    """Build the shared weight arrays (bf16 where needed)."""
    D = DIM
    scale = np.float32(D ** -0.5)
    wk_eff = (wk * g_in[:, None]).astype(np.float32)
    wq_eff = (wq * g_sl[:, None]).astype(np.float32)
    W_qk = (wk_eff @ wq_eff.T) * scale          # [D(x), D(sn)]
    W_qkT = np.ascontiguousarray(W_qk.T)        # [D(sn), D(x)] = lhsT
    w1row = np.ones(D, np.float32) @ W_qk       # [D(sn)]
    wv_eff = (wv * g_in[:, None]).astype(np.float32)
    wv1 = np.ones(D, np.float32) @ wv_eff       # [D]
    mlp1_eff = (mlp_w1 * g_ff[:, None]).astype(np.float32)

    def as_bf(x):
        return np.ascontiguousarray(x).astype(BF16)

    weights = {
        "wqkT": as_bf(W_qkT.reshape(2, 128, D)),
        "w1rT": as_bf(w1row.reshape(2, 128, 1)),
        "wv": as_bf(wv_eff.reshape(2, 128, D)),
        "wv1": np.ascontiguousarray(wv1.reshape(1, D)).astype(np.float32),
        "wih": as_bf(w_ih.reshape(2, 128, 3 * D)),
        "whh": as_bf(w_hh.reshape(2, 128, 3 * D)),
        "mlp1": as_bf(mlp1_eff.reshape(2, 128, HID)),
        "mlp2": as_bf(mlp_w2.reshape(8, 128, D)),
        "bsum": np.ascontiguousarray(
            (b_ih + b_hh)[:2 * D].reshape(4, 128).T).astype(np.float32),
        "bihn": np.ascontiguousarray(
            b_ih[2 * D:].reshape(2, 128).T).astype(np.float32),
        "bhhn": np.ascontiguousarray(
            b_hh[2 * D:].reshape(2, 128).T).astype(np.float32),
    }
    return weights


_NC_CACHE = {}


def kernel(inputs, slots, wq, bq, wk, bk, wv, bv, w_ih, b_ih, w_hh, b_hh,
           mlp_w1, mlp_b1, mlp_w2, mlp_b2, g_in, b_in, g_sl, b_sl,
           g_ff, b_ff, _trace=False, _trace_kwargs=None):
    from concourse import bass_utils

    inputs = np.asarray(inputs, np.float32)
    slots = np.asarray(slots, np.float32)
    B = inputs.shape[0]
    S, D = NUM_SLOTS, DIM

    # The reference setup always provides zero biases and unit gains for
    # these (verified); the kernel folds gains into weights and would need
    # extra bias terms otherwise.
    for z in (bq, bk, bv, b_in, b_sl, b_ff, mlp_b1, mlp_b2):
        assert np.abs(np.asarray(z)).max() == 0.0, "nonzero bias unsupported"

    weights = _host_prep(inputs, slots, np.asarray(wq), np.asarray(wk),
                         np.asarray(wv), np.asarray(w_ih), np.asarray(b_ih),
                         np.asarray(w_hh), np.asarray(b_hh),
                         np.asarray(mlp_w1), np.asarray(mlp_w2),
                         np.asarray(g_in), np.asarray(g_sl), np.asarray(g_ff))

    if "nc" not in _NC_CACHE:
        _NC_CACHE["nc"] = _build_nc()
    nc = _NC_CACHE["nc"]

    in_maps = []
    for c in range(N_CORES):
        xs = inputs[c * BL:(c + 1) * BL]                       # [BL, N, D]
        xbf = xs.astype(BF16).reshape(BL, 128, NCH, D)         # token 32p+c
        xnat = np.ascontiguousarray(xbf)
        # xt columns ordered (c, p): xt[:, d, c*128+p] = x[32p+c, d]
        xtb = np.ascontiguousarray(
            xbf.transpose(0, 3, 2, 1).reshape(BL, 2, 128, N))
        sl = np.ascontiguousarray(
            slots[:, c * BL:(c + 1) * BL, :].transpose(1, 0, 2)
            .reshape(BL * S, D)).astype(np.float32)
        m = {"xnat": xnat, "xt": xtb, "slots_in": sl}
        m.update(weights)
        in_maps.append(m)

    res = bass_utils.run_bass_kernel_spmd(nc, in_maps,
                                          core_ids=list(range(N_CORES)),
                                          trace=_trace,
                                          **(_trace_kwargs or {}))
    out = np.empty((S, B, D), np.float32)
    for c in range(N_CORES):
        o = res.results[c]["out"].reshape(BL, S, D)            # [b_local, s, d]
        out[:, c * BL:(c + 1) * BL, :] = o.transpose(1, 0, 2)
    kernel.last_results = res
    return out


if __name__ == "__main__":
    inp = np.load('/tmp/inp.npy', allow_pickle=True).item()
    expected = np.load('/tmp/expected.npy')
    actual = kernel(**{k: np.asarray(v) for k, v in inp.items()})
    rel = np.abs(actual - expected).max() / (np.abs(expected).max() + 1e-12)
    print(f"Relative error: {rel:.3e}")


# revision 22
# speedup vs baseline: 1.0408x; 1.0408x over previous
"""Slot-attention module Bass/Tile kernel (nn_AttentionModule_39084202394083).

kernel(**inputs) takes FULL unsharded inputs, shards batch B=64 across 8
NeuronCores (8 batch elements per core), runs a Bass/Tile kernel per core,
and gathers the FULL output [S=8, B=64, D=256] fp32.

Device algorithm:
- All matmuls bf16 (fp32 PSUM accumulation); correctness gate is 2e-2.
- Input LN is folded into the attention algebra: per-token stats come from
  bn_stats on natural-layout x; the (x-m)*r normalize is never
  materialized. The mean enters dots via an augmented contraction row
  (mT row x -colsum(M_k)) and enters updates via an extra value column;
  the 1/std scale r enters dots pre-exp and updates via the softmax
  weights. Host passes a pre-transposed bf16 x (stationary operand).
- wq/wk are pre-multiplied on host (W_qk = wk @ wq.T * D^-0.5) so q/k are
  never materialized; logits for 128-token chunks are x_chunk.T @ (W_qk@snT).
- Softmax over the slot axis runs in token-major layout [128, 32*8].
- LN rsqrt is computed on VectorE (bit-trick seed + 2 Newton steps) so
  ScalarE only ever uses the exp/tanh table set (no table thrash).
- GRU + MLP run in transposed layout [D, rows] with weights stationary;
  sigmoid is computed as 0.5 + 0.5*tanh(x/2).
"""

import numpy as np
import ml_dtypes

BF16 = ml_dtypes.bfloat16


def _install_ntff_hook_shim():
    """Provide antenv.axon_hooks in containers whose antenv stub lacks it,
    backed by direct ctypes calls into libaxon_pjrt.so (same recipe as
    trn_agent_boot). Enables trace=True HW timing via run_bass_kernel_spmd."""
    import sys, types, ctypes, contextlib, os
    try:
        from antenv.axon_hooks import get_axon_ntff_profile_hook  # noqa
        return
    except ImportError:
        pass
    so_path = "/opt/axon/libaxon_pjrt.so"
    if not os.path.exists(so_path):
        return
    try:
        lib = ctypes.CDLL(so_path)
        if not hasattr(lib, "axon_start_nrt_profile"):
            return
        lib.axon_start_nrt_profile.argtypes = [
            ctypes.POINTER(ctypes.c_int64), ctypes.c_size_t]
        lib.axon_start_nrt_profile.restype = ctypes.c_int64
        lib.axon_stop_nrt_profile.argtypes = [ctypes.c_char_p]
        lib.axon_stop_nrt_profile.restype = ctypes.c_int64
    except OSError:
        return

    @contextlib.contextmanager
    def _hook(output_dir, device_ids):
        import jax
        jax.devices()
        if device_ids:
            ids = (ctypes.c_int64 * len(device_ids))(*device_ids)
            rc = lib.axon_start_nrt_profile(ids, len(device_ids))
        else:
            rc = lib.axon_start_nrt_profile(None, 0)
        if rc != 0:
            raise RuntimeError(f"axon_start_nrt_profile rc={rc}")
        try:
            yield
        finally:
            n = lib.axon_stop_nrt_profile(str(output_dir).encode())
            if n < 0:
                raise RuntimeError(f"axon_stop_nrt_profile rc={n}")

    mod = types.ModuleType("antenv.axon_hooks")
    mod.get_axon_ntff_profile_hook = lambda: _hook
    mod.set_axon_ntff_profile_hook = lambda h: None
    import antenv
    antenv.axon_hooks = mod
    sys.modules["antenv.axon_hooks"] = mod


_install_ntff_hook_shim()

NUM_SLOTS = 8
DIM = 256
HID = 1024
ITERS = 3
LN_EPS = 1e-05

N_CORES = 8
BL = 8          # batch elements per core
GB = 2          # batch elements per phase-group
NGROUPS = BL // GB
N = 4096        # tokens
NCH = N // 128  # token chunks of 128
ROWS = GB * NUM_SLOTS  # slot-rows per group (16)

_RSQRT_MAGIC = 0x5F3759DF


def _build_nc():
    import concourse.bass as bass
    import concourse.bacc as bacc
    import concourse.tile as tile
    from concourse import mybir

    f32 = mybir.dt.float32
    bf16 = mybir.dt.bfloat16
    i32 = mybir.dt.int32
    AX = mybir.AxisListType
    OP = mybir.AluOpType
    ACT = mybir.ActivationFunctionType

    nc = bacc.Bacc("TRN2", target_bir_lowering=False)

    xnat = nc.dram_tensor("xnat", [BL, 128, NCH, DIM], bf16, kind="ExternalInput")
    xt = nc.dram_tensor("xt", [BL, 2, 128, N], bf16, kind="ExternalInput")
    slots_in = nc.dram_tensor("slots_in", [BL * NUM_SLOTS, DIM], f32,
                              kind="ExternalInput")
    wqkT_d = nc.dram_tensor("wqkT", [2, 128, DIM], bf16, kind="ExternalInput")
    w1rT_d = nc.dram_tensor("w1rT", [2, 128, 1], bf16, kind="ExternalInput")
    wv_d = nc.dram_tensor("wv", [2, 128, DIM], bf16, kind="ExternalInput")
    wv1_d = nc.dram_tensor("wv1", [1, DIM], f32, kind="ExternalInput")
    wih_d = nc.dram_tensor("wih", [2, 128, 3 * DIM], bf16, kind="ExternalInput")
    whh_d = nc.dram_tensor("whh", [2, 128, 3 * DIM], bf16, kind="ExternalInput")
    mlp1_d = nc.dram_tensor("mlp1", [2, 128, HID], bf16, kind="ExternalInput")
    mlp2_d = nc.dram_tensor("mlp2", [8, 128, DIM], bf16, kind="ExternalInput")
    bsum_d = nc.dram_tensor("bsum", [128, 4], f32, kind="ExternalInput")
    bihn_d = nc.dram_tensor("bihn", [128, 2], f32, kind="ExternalInput")
    bhhn_d = nc.dram_tensor("bhhn", [128, 2], f32, kind="ExternalInput")
    out_d = nc.dram_tensor("out", [BL * NUM_SLOTS, DIM], f32,
                           kind="ExternalOutput")

    with tile.TileContext(nc) as tc:
        with (
            tc.tile_pool(name="consts", bufs=1) as consts,
            tc.tile_pool(name="xtp", bufs=1) as xtp,
            tc.tile_pool(name="vpool", bufs=1) as vpool,
            tc.tile_pool(name="perb", bufs=1) as perb,
            tc.tile_pool(name="xch", bufs=3) as xch_pool,
            tc.tile_pool(name="stats", bufs=2) as stats,
            tc.tile_pool(name="sm", bufs=2) as sm,
            tc.tile_pool(name="ep", bufs=2) as ep,
            tc.tile_pool(name="slp", bufs=1) as slp,
            tc.tile_pool(name="ps", bufs=1, space="PSUM") as ps,
            tc.tile_pool(name="ps2", bufs=2, space="PSUM") as ps2,
        ):
            from concourse.masks import make_identity
            ident = consts.tile([128, 128], bf16, tag="ident")
            make_identity(nc, ident[:])
            identf = consts.tile([128, 128], f32, tag="identf")
            make_identity(nc, identf[:])

            wqk_sb = consts.tile([128, 2, DIM], bf16, tag="wqk")
            w1r_sb = consts.tile([128, 2, 1], bf16, tag="w1r")
            wv_sb = consts.tile([128, 2, DIM], bf16, tag="wv")
            wih_sb = consts.tile([128, 2, 3 * DIM], bf16, tag="wih")
            whh_sb = consts.tile([128, 2, 3 * DIM], bf16, tag="whh")
            mlp1_sb = consts.tile([128, 2, HID], bf16, tag="mlp1")
            mlp2_sb = consts.tile([128, 8, DIM], bf16, tag="mlp2")
            for _kt in range(2):
                nc.sync.dma_start(wqk_sb[:, _kt, :], wqkT_d[_kt])
                nc.sync.dma_start(w1r_sb[:, _kt, :], w1rT_d[_kt])
                nc.sync.dma_start(wv_sb[:, _kt, :], wv_d[_kt])
                nc.sync.dma_start(wih_sb[:, _kt, :], wih_d[_kt])
                nc.sync.dma_start(whh_sb[:, _kt, :], whh_d[_kt])
                nc.sync.dma_start(mlp1_sb[:, _kt, :], mlp1_d[_kt])
            for _kt in range(8):
                nc.sync.dma_start(mlp2_sb[:, _kt, :], mlp2_d[_kt])
            bsum_sb = consts.tile([128, 4], f32, tag="bsum")
            nc.sync.dma_start(bsum_sb[:], bsum_d[:, :])
            bihn_sb = consts.tile([128, 2], f32, tag="bihn")
            nc.sync.dma_start(bihn_sb[:], bihn_d[:, :])
            bhhn_sb = consts.tile([128, 2], f32, tag="bhhn")
            nc.sync.dma_start(bhhn_sb[:], bhhn_d[:, :])
            ones_row = consts.tile([1, 128], bf16, tag="ones_row")
            nc.gpsimd.memset(ones_row[:], 1.0)
            wv1_sb = consts.tile([128, DIM], f32, tag="wv1")
            _w = wv1_d[:, :]
            wv1_bc = bass.AP(tensor=_w.tensor, offset=_w.offset,
                             ap=[[0, 128]] + list(_w.ap)[1:])
            nc.sync.dma_start(wv1_sb[:], wv1_bc)

            def bcast8(ap2):
                """[128, NCH] AP -> [128, NCH, 8] stride-0 broadcast."""
                return bass.AP(tensor=ap2.tensor, offset=ap2.offset,
                               ap=list(ap2.ap) + [[0, 8]])

            def newton_rsqrt(eng, ve_ap, y_ap, u_ap):
                """y = 1/sqrt(ve), bit-trick seed + 2 Newton steps (f32)."""
                vi = ve_ap.bitcast(i32)
                yi = y_ap.bitcast(i32)
                eng.tensor_scalar(yi, vi, 1, None, OP.arith_shift_right)
                eng.tensor_scalar(yi, yi, -1, _RSQRT_MAGIC, OP.mult, OP.add)
                for _ in range(2):
                    eng.tensor_tensor(u_ap, y_ap, y_ap, OP.mult)
                    eng.tensor_tensor(u_ap, u_ap, ve_ap, OP.mult)
                    eng.tensor_scalar(u_ap, u_ap, -0.5, 1.5, OP.mult, OP.add)
                    eng.tensor_tensor(y_ap, y_ap, u_ap, OP.mult)

            def transpose_in(dst, src, rows):
                """dst [128, 2, rows] bf16 <- src [rows, 256] bf16."""
                for kt in range(2):
                    tp = ps2.tile([128, 128], bf16, tag="tp")
                    nc.tensor.transpose(tp[:128, :rows],
                                        src[:, kt * 128:(kt + 1) * 128],
                                        ident[0:rows, 0:rows])
                    nc.vector.tensor_copy(dst[:, kt, :], tp[:128, :rows])

            def transpose_out(dst, srcT, rows):
                """dst [rows, 256] f32 <- srcT [128, 2, rows] f32."""
                for kt in range(2):
                    tp = ps2.tile([128, 128], f32, tag="tp")
                    nc.tensor.transpose(tp[:rows, :128], srcT[:, kt, :],
                                        identf[:, :])
                    nc.vector.tensor_copy(dst[:, kt * 128:(kt + 1) * 128],
                                          tp[:rows, :128])

            def phase_sv(g, gi):
                """Load xt, compute stats + v for group g (slot gi)."""
                st = {"xt": [], "y": [], "mr2": [], "v": []}
                for b in range(GB):
                    gb = g * GB + b
                    tiles = []
                    for kt in range(2):
                        t = xtp.tile([128, N], bf16, tag=f"xt{gi}_{b}_{kt}")
                        nc.sync.dma_start(t[:], xt[gb, kt])
                        tiles.append(t)
                    st["xt"].append(tiles)

                    mv = stats.tile([128, NCH, 2], f32, tag="mv")
                    for j4 in range(4):
                        xb = xch_pool.tile([128, 8, DIM], bf16, tag="xc")
                        nc.sync.dma_start(xb[:],
                                          xnat[gb, :, j4 * 8:(j4 + 1) * 8, :])
                        for jj in range(8):
                            c = j4 * 8 + jj
                            st6 = sm.tile([128, 6], f32, tag="st6")
                            nc.vector.bn_stats(st6[:], xb[:, jj, :])
                            nc.vector.bn_aggr(mv[:, c, :], st6[:])
                    ve = stats.tile([128, NCH], f32, tag="ve")
                    nc.vector.tensor_scalar(ve[:], mv[:, :, 1], LN_EPS, None,
                                            OP.add)
                    y = perb.tile([128, NCH], f32, tag=f"y{gi}{b}")
                    u = sm.tile([128, NCH], f32, tag="u")
                    newton_rsqrt(nc.vector, ve[:], y[:], u[:])
                    st["y"].append(y)
                    srec = sm.tile([128, NCH], f32, tag="srec")
                    nc.vector.tensor_tensor(srec[:], ve[:], y[:], OP.mult)
                    m_bf = stats.tile([128, NCH], bf16, tag="mbf")
                    nc.gpsimd.tensor_copy(m_bf[:], mv[:, :, 0])
                    mr2 = perb.tile([128, NCH], f32, tag=f"mr2{gi}{b}")
                    nc.vector.tensor_tensor(mr2[:], mv[:, :, 0], y[:],
                                            OP.mult)
                    st["mr2"].append(mr2)

                    vt = vpool.tile([128, NCH, DIM + 2], bf16, tag=f"v{gi}{b}")
                    nc.gpsimd.tensor_copy(vt[:, :, DIM], srec[:])
                    nc.gpsimd.tensor_copy(vt[:, :, DIM + 1], m_bf[:])
                    for cc in range(0, NCH, 2):
                        vps = ps2.tile([128, 2, DIM], f32, tag="vp")
                        for j in range(2):
                            c = cc + j
                            for kt in range(2):
                                nc.tensor.matmul(
                                    vps[:, j, :],
                                    tiles[kt][:, c * 128:(c + 1) * 128],
                                    wv_sb[:, kt, :],
                                    start=(kt == 0), stop=(kt == 1))
                        nc.scalar.activation(vt[:, cc:cc + 2, 0:DIM], vps[:],
                                             ACT.Copy)
                    st["v"].append(vt)

                sl_nat = slp.tile([ROWS, DIM], f32, tag=f"slnat{gi}")
                nc.sync.dma_start(sl_nat[:],
                                  slots_in[g * ROWS:(g + 1) * ROWS, :])
                sl_bf0 = slp.tile([ROWS, DIM], bf16, tag=f"slbf0{gi}")
                nc.vector.tensor_copy(sl_bf0[:], sl_nat[:])
                slT_bf = slp.tile([128, 2, ROWS], bf16, tag=f"slTbf{gi}")
                transpose_in(slT_bf, sl_bf0, ROWS)
                st["sl_nat"] = sl_nat
                st["slT_bf"] = slT_bf
                return st

            def ln0(src, rows, out_bf, eng=None):
                """out_bf [rows, 256] bf16 = LN0(src [rows, 256])."""
                eng = nc.vector
                st6 = sm.tile([rows, 6], f32, tag="st6b")
                nc.vector.bn_stats(st6[:], src[:])
                mv2 = sm.tile([rows, 2], f32, tag="mv2")
                nc.vector.bn_aggr(mv2[:], st6[:])
                ve2 = sm.tile([rows, 1], f32, tag="ve2")
                eng.tensor_scalar(ve2[:], mv2[:, 1:2], LN_EPS, None, OP.add)
                y2 = sm.tile([rows, 1], f32, tag="y2")
                u2 = sm.tile([rows, 1], f32, tag="u2")
                newton_rsqrt(eng, ve2[:], y2[:], u2[:])
                nc.vector.tensor_scalar(out_bf[:], src[:], mv2[:, 0:1],
                                        y2[:], OP.subtract, OP.mult)

            def iteration(g, gi, it, st):
                # ---- P1: sn = LN0(slots), snT ----
                sn_bf = sm.tile([ROWS, DIM], bf16, tag="snbf")
                ln0(st["sl_nat"], ROWS, sn_bf)
                snT = sm.tile([128, 2, ROWS], bf16, tag="snT")
                transpose_in(snT, sn_bf, ROWS)
                # ---- P3: M_k, Mksum broadcast ----
                mk_ps = ps2.tile([128, 2, ROWS], f32, tag="tp")
                for mt in range(2):
                    for kt in range(2):
                        nc.tensor.matmul(
                            mk_ps[:, mt, :],
                            wqk_sb[:, kt, mt * 128:(mt + 1) * 128],
                            snT[:, kt, :], start=(kt == 0), stop=(kt == 1))
                mk_sb = sm.tile([128, 2, ROWS], bf16, tag="mksb")
                nc.vector.tensor_copy(mk_sb[:], mk_ps[:])
                mn_ps = ps2.tile([1, ROWS], f32, tag="tp")
                for kt in range(2):
                    nc.tensor.matmul(mn_ps[:], w1r_sb[:, kt, :],
                                     snT[:, kt, :],
                                     start=(kt == 0), stop=(kt == 1))
                mks_bf = sm.tile([1, ROWS], bf16, tag="mksbf")
                nc.vector.tensor_copy(mks_bf[:], mn_ps[:])
                bc_ps = ps2.tile([128, ROWS], f32, tag="tp")
                nc.tensor.matmul(bc_ps[:], ones_row[:], mks_bf[:],
                                 start=True, stop=True)
                mkbc = sm.tile([128, ROWS], f32, tag="mkbc")
                nc.vector.tensor_copy(mkbc[:], bc_ps[:])

                # ---- P4: attention per batch ----
                upd_ps = ps2.tile([128, DIM + 2], f32, tag="vp")
                for b in range(GB):
                    brow = b * NUM_SLOTS
                    bsl = slice(brow, brow + NUM_SLOTS)
                    dt_ps = ps2.tile([128, NCH, NUM_SLOTS], f32, tag="dt")
                    for c in range(NCH):
                        cs = slice(c * 128, (c + 1) * 128)
                        nc.tensor.matmul(dt_ps[:, c, :],
                                         st["xt"][b][0][:, cs],
                                         mk_sb[:, 0, bsl],
                                         start=True, stop=False)
                        nc.tensor.matmul(dt_ps[:, c, :],
                                         st["xt"][b][1][:, cs],
                                         mk_sb[:, 1, bsl],
                                         start=False, stop=True)
                    _mk = mkbc[:, bsl]
                    mk_bc = bass.AP(tensor=_mk.tensor, offset=_mk.offset,
                                    ap=[_mk.ap[0], [0, NCH], _mk.ap[1]])
                    t2r = ep.tile([128, NCH, 8], f32, tag="t2r")
                    nc.vector.tensor_tensor(t2r[:], bcast8(st["mr2"][b][:]),
                                            mk_bc, OP.mult)
                    tmp_e = ep.tile([128, NCH, 8], f32, tag="tmpe")
                    nc.vector.tensor_tensor(tmp_e[:], dt_ps[:],
                                            bcast8(st["y"][b][:]), OP.mult)
                    et_f = ep.tile([128, NCH, 8], f32, tag="etf")
                    nc.vector.tensor_tensor(et_f[:], tmp_e[:], t2r[:],
                                            OP.subtract)
                    eT = ep.tile([128, NCH, 8], bf16, tag="eT")
                    nc.scalar.activation(
                        eT[:].rearrange("p a b -> p (a b)"),
                        et_f[:].rearrange("p a b -> p (a b)"), ACT.Exp)
                    cs_f = sm.tile([128, NCH], f32, tag="csf")
                    nc.vector.tensor_reduce(cs_f[:], eT[:], axis=AX.X,
                                            op=OP.add)
                    rc = sm.tile([128, NCH], f32, tag="rc")
                    nc.vector.reciprocal(rc[:], cs_f[:])
                    rcr = sm.tile([128, NCH], f32, tag="rcr")
                    nc.vector.tensor_tensor(rcr[:], rc[:], st["y"][b][:],
                                            OP.mult)
                    wpT = ep.tile([128, NCH, 8], bf16, tag="wpT")
                    nc.vector.tensor_tensor(wpT[:], eT[:], bcast8(rcr[:]),
                                            OP.mult)
                    for c in range(NCH):
                        nc.tensor.matmul(upd_ps[32 * b:32 * b + NUM_SLOTS, :],
                                         wpT[:, c, :], st["v"][b][:, c, :],
                                         start=(c == 0), stop=(c == NCH - 1),
                                         tile_position=(0, 32 * b))

                # ---- P5: updates fixup + GRU ----
                upd_bf = sm.tile([128, DIM], bf16, tag="updbf")
                rn_all = sm.tile([128, 1], f32, tag="rn")
                murn_all = sm.tile([128, 1], f32, tag="murn")
                tfix_all = sm.tile([128, DIM], f32, tag="tfix")
                nc.vector.memset(upd_bf[:], 0.0)
                for b in range(GB):
                    psl = slice(32 * b, 32 * b + NUM_SLOTS)
                    nc.vector.reciprocal(rn_all[psl],
                                         upd_ps[psl, DIM:DIM + 1])
                    nc.vector.tensor_tensor(
                        murn_all[psl], upd_ps[psl, DIM + 1:DIM + 2],
                        rn_all[psl], OP.mult)
                    nc.vector.tensor_scalar(tfix_all[psl], wv1_sb[psl],
                                            murn_all[psl], None, OP.mult)
                    nc.vector.scalar_tensor_tensor(
                        upd_bf[psl], upd_ps[psl, 0:DIM], rn_all[psl],
                        tfix_all[psl], OP.mult, OP.subtract)
                updT = sm.tile([128, 2, 128], bf16, tag="updT")
                for kt in range(2):
                    tpu = ps2.tile([128, 128], bf16, tag="tp")
                    nc.tensor.transpose(tpu[:],
                                        upd_bf[:, kt * 128:(kt + 1) * 128],
                                        ident[:, :])
                    nc.vector.tensor_copy(updT[:, kt, :], tpu[:])

                gx_ps = ps.tile([128, 6, ROWS], f32, tag="gx")
                gh_ps = ps.tile([128, 6, ROWS], f32, tag="gh")
                for t in range(6):
                    tsl = slice(t * 128, (t + 1) * 128)
                    for kt in range(2):
                        nc.tensor.matmul(
                            gx_ps[:, t, :], wih_sb[:, kt, tsl],
                            updT[:, kt, :]
                            .rearrange("p (b s) -> p b s", s=32)
                            [:, 0:GB, 0:NUM_SLOTS],
                            start=(kt == 0), stop=(kt == 1))
                        nc.tensor.matmul(
                            gh_ps[:, t, :], whh_sb[:, kt, tsl],
                            st["slT_bf"][:, kt, :],
                            start=(kt == 0), stop=(kt == 1))
                gh_sb = sm.tile([128, 6, ROWS], f32, tag="ghsb")
                nc.scalar.activation(
                    gh_sb[:].rearrange("p a b -> p (a b)"),
                    gh_ps[:].rearrange("p a b -> p (a b)"),
                    ACT.Copy)
                rzp = sm.tile([128, 4, ROWS], f32, tag="rzp")
                for t in range(4):
                    nc.vector.scalar_tensor_tensor(
                        rzp[:, t, :], gx_ps[:, t, :], bsum_sb[:, t:t + 1],
                        gh_sb[:, t, :], OP.add, OP.add)
                rz_t = sm.tile([128, 4, ROWS], f32, tag="rzt")
                nc.scalar.activation(
                    rz_t[:].rearrange("p a b -> p (a b)"),
                    rzp[:].rearrange("p a b -> p (a b)"), ACT.Tanh, scale=0.5)
                rz = sm.tile([128, 4, ROWS], bf16, tag="rz")
                nc.vector.tensor_scalar(rz[:], rz_t[:], 0.5, 0.5, OP.mult,
                                        OP.add)
                hnp = sm.tile([128, 2, ROWS], f32, tag="hnp")
                for t in range(2):
                    nc.vector.tensor_scalar(hnp[:, t, :], gh_sb[:, 4 + t, :],
                                            bhhn_sb[:, t:t + 1], None, OP.add)
                t2t = sm.tile([128, 2, ROWS], f32, tag="t2t")
                nc.vector.tensor_tensor(t2t[:], rz[:, 0:2, :], hnp[:],
                                        OP.mult)
                npre = sm.tile([128, 2, ROWS], f32, tag="npre")
                for t in range(2):
                    nc.vector.scalar_tensor_tensor(
                        npre[:, t, :], gx_ps[:, 4 + t, :],
                        bihn_sb[:, t:t + 1], t2t[:, t, :], OP.add, OP.add)
                nT = sm.tile([128, 2, ROWS], bf16, tag="nT")
                nc.scalar.activation(
                    nT[:].rearrange("p a b -> p (a b)"),
                    npre[:].rearrange("p a b -> p (a b)"), ACT.Tanh)
                dd = sm.tile([128, 2, ROWS], f32, tag="dd")
                nc.vector.tensor_tensor(dd[:], st["slT_bf"][:], nT[:],
                                        OP.subtract)
                zd = sm.tile([128, 2, ROWS], f32, tag="zd")
                nc.vector.tensor_tensor(zd[:], rz[:, 2:4, :], dd[:], OP.mult)
                smidT_f = slp.tile([128, 2, ROWS], f32, tag=f"smidTf{gi}")
                nc.vector.tensor_tensor(smidT_f[:], nT[:], zd[:], OP.add)

                # ---- P6: LN-ff + MLP + residual ----
                smid_nat = sm.tile([ROWS, DIM], f32, tag="smidnat")
                transpose_out(smid_nat, smidT_f, ROWS)
                ffn_bf = sm.tile([ROWS, DIM], bf16, tag="ffnbf")
                ln0(smid_nat, ROWS, ffn_bf, eng=nc.gpsimd)
                ffnT = sm.tile([128, 2, ROWS], bf16, tag="ffnT")
                transpose_in(ffnT, ffn_bf, ROWS)
                h1_ps = ps2.tile([128, 8, ROWS], f32, tag="vp")
                for t in range(8):
                    tsl = slice(t * 128, (t + 1) * 128)
                    for kt in range(2):
                        nc.tensor.matmul(h1_ps[:, t, :], mlp1_sb[:, kt, tsl],
                                         ffnT[:, kt, :],
                                         start=(kt == 0), stop=(kt == 1))
                h1_bf = sm.tile([128, 8, ROWS], bf16, tag="h1bf")
                nc.vector.tensor_scalar(
                    h1_bf[:].rearrange("p a b -> p (a b)"),
                    h1_ps[:].rearrange("p a b -> p (a b)"), 0.0, None, OP.max)
                o2_ps = ps.tile([128, 2, ROWS], f32, tag="gx")
                for mt in range(2):
                    msl = slice(mt * 128, (mt + 1) * 128)
                    for kt in range(8):
                        nc.tensor.matmul(o2_ps[:, mt, :], mlp2_sb[:, kt, msl],
                                         h1_bf[:, kt, :],
                                         start=(kt == 0), stop=(kt == 7))
                slT_new_f = slp.tile([128, 2, ROWS], f32, tag=f"slTnf{gi}")
                nc.vector.tensor_tensor(slT_new_f[:], smidT_f[:], o2_ps[:],
                                        OP.add)
                new_nat = slp.tile([ROWS, DIM], f32, tag=f"slnat2{gi}")
                transpose_out(new_nat, slT_new_f, ROWS)
                st["sl_nat"] = new_nat
                if it < ITERS - 1:
                    slT_bf = slp.tile([128, 2, ROWS], bf16, tag=f"slTbf{gi}")
                    nc.vector.tensor_copy(slT_bf[:], slT_new_f[:])
                    st["slT_bf"] = slT_bf
                else:
                    nc.sync.dma_start(out_d[g * ROWS:(g + 1) * ROWS, :],
                                      new_nat[:])

            for pair in range(NGROUPS // 2):
                sts = []
                for gi in range(2):
                    sts.append(phase_sv(2 * pair + gi, gi))
                for it in range(ITERS):
                    for gi in range(2):
                        iteration(2 * pair + gi, gi, it, sts[gi])

    nc.compile()
    return nc


def _host_prep(inputs, slots, wq, wk, wv, w_ih, b_ih, w_hh, b_hh,
               mlp_w1, mlp_w2, g_in, g_sl, g_ff):
    """Build the shared weight arrays (bf16 where needed)."""
    D = DIM
    scale = np.float32(D ** -0.5)
    wk_eff = (wk * g_in[:, None]).astype(np.float32)
    wq_eff = (wq * g_sl[:, None]).astype(np.float32)
    W_qk = (wk_eff @ wq_eff.T) * scale          # [D(x), D(sn)]
    W_qkT = np.ascontiguousarray(W_qk.T)        # [D(sn), D(x)] = lhsT
    w1row = np.ones(D, np.float32) @ W_qk       # [D(sn)]
    wv_eff = (wv * g_in[:, None]).astype(np.float32)
    wv1 = np.ones(D, np.float32) @ wv_eff       # [D]
    mlp1_eff = (mlp_w1 * g_ff[:, None]).astype(np.float32)

    def as_bf(x):
        return np.ascontiguousarray(x).astype(BF16)

    weights = {
        "wqkT": as_bf(W_qkT.reshape(2, 128, D)),
        "w1rT": as_bf(w1row.reshape(2, 128, 1)),
        "wv": as_bf(wv_eff.reshape(2, 128, D)),
        "wv1": np.ascontiguousarray(wv1.reshape(1, D)).astype(np.float32),
        "wih": as_bf(w_ih.reshape(2, 128, 3 * D)),
        "whh": as_bf(w_hh.reshape(2, 128, 3 * D)),
        "mlp1": as_bf(mlp1_eff.reshape(2, 128, HID)),
        "mlp2": as_bf(mlp_w2.reshape(8, 128, D)),
        "bsum": np.ascontiguousarray(
            (b_ih + b_hh)[:2 * D].reshape(4, 128).T).astype(np.float32),
        "bihn": np.ascontiguousarray(
            b_ih[2 * D:].reshape(2, 128).T).astype(np.float32),
        "bhhn": np.ascontiguousarray(
            b_hh[2 * D:].reshape(2, 128).T).astype(np.float32),
    }
    return weights


_NC_CACHE = {}


def kernel(inputs, slots, wq, bq, wk, bk, wv, bv, w_ih, b_ih, w_hh, b_hh,
           mlp_w1, mlp_b1, mlp_w2, mlp_b2, g_in, b_in, g_sl, b_sl,
           g_ff, b_ff, _trace=False, _trace_kwargs=None):
    from concourse import bass_utils

    inputs = np.asarray(inputs, np.float32)
    slots = np.asarray(slots, np.float32)
    B = inputs.shape[0]
    S, D = NUM_SLOTS, DIM

    # The reference setup always provides zero biases and unit gains for
    # these (verified); the kernel folds gains into weights and would need
    # extra bias terms otherwise.
    for z in (bq, bk, bv, b_in, b_sl, b_ff, mlp_b1, mlp_b2):
        assert np.abs(np.asarray(z)).max() == 0.0, "nonzero bias unsupported"

    weights = _host_prep(inputs, slots, np.asarray(wq), np.asarray(wk),
                         np.asarray(wv), np.asarray(w_ih), np.asarray(b_ih),
                         np.asarray(w_hh), np.asarray(b_hh),
                         np.asarray(mlp_w1), np.asarray(mlp_w2),
                         np.asarray(g_in), np.asarray(g_sl), np.asarray(g_ff))

    if "nc" not in _NC_CACHE:
        _NC_CACHE["nc"] = _build_nc()
    nc = _NC_CACHE["nc"]

    in_maps = []
    for c in range(N_CORES):
        xs = inputs[c * BL:(c + 1) * BL]                       # [BL, N, D]
        xbf = xs.astype(BF16).reshape(BL, 128, NCH, D)         # token 32p+c
        xnat = np.ascontiguousarray(xbf)
        # xt columns ordered (c, p): xt[:, d, c*128+p] = x[32p+c, d]
        xtb = np.ascontiguousarray(
            xbf.transpose(0, 3, 2, 1).reshape(BL, 2, 128, N))
        sl = np.ascontiguousarray(
            slots[:, c * BL:(c + 1) * BL, :].transpose(1, 0, 2)
            .reshape(BL * S, D)).astype(np.float32)
        m = {"xnat": xnat, "xt": xtb, "slots_in": sl}
        m.update(weights)
        in_maps.append(m)

    res = bass_utils.run_bass_kernel_spmd(nc, in_maps,
                                          core_ids=list(range(N_CORES)),
                                          trace=_trace,
                                          **(_trace_kwargs or {}))
    out = np.empty((S, B, D), np.float32)
    for c in range(N_CORES):
        o = res.results[c]["out"].reshape(BL, S, D)            # [b_local, s, d]
        out[:, c * BL:(c + 1) * BL, :] = o.transpose(1, 0, 2)
    kernel.last_results = res
    return out


if __name__ == "__main__":
    inp = np.load('/tmp/inp.npy', allow_pickle=True).item()
    expected = np.load('/tmp/expected.npy')
    actual = kernel(**{k: np.asarray(v) for k, v in inp.items()})
    rel = np.abs(actual - expected).max() / (np.abs(expected).max() + 1e-12)
    print(f"Relative error: {rel:.3e}")
